# revision 38
# baseline (speedup 1.0000x reference)
"""Trainium2 Bass kernel for nn_ErdosLoss (graph loss function).

Math (reference reformulated, validated to ~1e-6 rel err):
  penalty:  log_score = scatter_add(log(1 - p + 1e-6), tgt)   over N nodes
            loss2 = mean(exp(log_score)) * 9600
  loss3:    p @ triu(H H^T, 1) @ p^T  ==  (||s||^2 - sum_e d_e p_e^2) / 2
            where s = scatter_add(p, tgt) + scatter_add(p, src),
            self-loop edges get -1 local_scatter indices (host-side index
            prep) so their src contribution vanishes; d_e = 2 - m_e is
            shipped as an index-derived weight column w.
  out = loss2 + 200 * loss3 / num_graphs,  num_graphs = max(batch) + 1.

Device strategy (8 NeuronCores, SPMD, two launches, no collectives):
  Per-NEFF measured window = first named inst -> trace end, and the trace
  end includes a fixed ~7.4us framework tail (the NEFF execution-loop
  preamble: each engine clears its ~51-semaphore block one inst at a
  time).  Per-launch serial anatomy: input DMA latency (trigger 565ns +
  HWDGE 625ns + delay 650ns + transfer + sem prop), compute, output DMA,
  pool teardown, tail.  Design choices driven by that:
  - The OUTPUT DMA is issued after the TileContext closes (raw staging
    buffer + explicit completion sem nobody waits on), so no pool barrier
    waits on it: the ~2us transfer overlaps the fixed tail.  (Issuing the
    INPUT DMAs pre-TileContext with manual sem waits was tried and hangs
    the NEFF on hardware — keep input DMAs tile-tracked.)
  - Phase 1 (8 cores, edge-sharded 750/core): scatter-add via one-hot
    matmul with node = 128*hi + lo decomposition (N padded to 4096).
    iota comes from a DVE prefix-scan of ones (no GPSIMD standard-lib
    dependency); the local_scatter Q7 library is swapped in as the first
    gpsimd op so its ~2.2us code DMA overlaps the input DMA.  Engine
    split:
    Vector builds H_tgt + A_tgt one-hots + RS (= H x [logmsg | p]) and
    the dp2 row-sum; GpSimd builds the src side via local_scatter (a 1
    at t*128+u_lo; p-f16 at t*32+u_hi), -1 indices killing self-loops
    and pads.  TensorE contracts into PSUM [128lo, 64] (= log_score | s).
  - Host gathers the 8 partials (pure data movement, c-innermost).
  - Phase 2 (1 core): 8-way reduces + exp/square row-sums (accum_out),
    f16 ones-matmul partition reduce, num_graphs = max(batch)+1 from the
    sorted batch tail, fused scalar chain.
  Engine-queue FIFO order is load-bearing: ops are emitted in
  critical-path order per engine.
"""

import numpy as np

import concourse.bacc as bacc
import concourse.mybir as mybir
import concourse.tile as tile
from concourse import bass_utils
from concourse import library_config

F32 = mybir.dt.float32
F16 = mybir.dt.float16
ALU = mybir.AluOpType
ACT = mybir.ActivationFunctionType
AX = mybir.AxisListType

N_NODES = 4000
N_EDGES = 6000
N_CORES = 8
N_PAD = 4096          # 128 * 32
HI = 32               # node hi-digits
LO = 128              # node lo-digits
PENALTY_SCALE = 16 * 200 * 3   # 9600
PAD_NODES = N_PAD - N_NODES    # 96 padded nodes, each contributes exp(0)=1

EPC = N_EDGES // N_CORES       # 750 edges per core
TPC = (EPC + 127) // 128       # 6 edge tiles per core


def _build_phase1(T: int):
    """Per-core partial computation: out 'partial' [128, 65] f16."""
    nc = bacc.Bacc("TRN2", target_bir_lowering=False, debug=False, num_devices=1)

    NC = 4 * T
    edatad = nc.dram_tensor("edata", [128, NC], F32, kind="ExternalInput").ap()
    eidx2d = nc.dram_tensor("eidx2", [128, 2 * T], mybir.dt.int16,
                            kind="ExternalInput").ap()
    partiald = nc.dram_tensor("partial", [128, 65], F16, kind="ExternalOutput").ap()

    # raw staging buffer for the post-TileContext output DMA
    C = nc.alloc_sbuf_tensor("C_out", [128, 65], F16).ap()
    odma_sem = nc.alloc_semaphore("odma_sem")

    with tile.TileContext(nc) as tc:
        with (
            tc.tile_pool(name="work", bufs=1) as wpool,
            tc.tile_pool(name="psum", bufs=1, space="PSUM") as ppool,
        ):
            # gpsimd: swap in the local_scatter Q7 library first — its
            # code DMA overlaps the input DMA latency
            nc.gpsimd.load_library(library_config.local_scatter)
            ed_t = wpool.tile([128, NC], F32, tag="ed_t")
            nc.sync.dma_start(ed_t[:], edatad)
            ei2_t = wpool.tile([128, 2 * T], mybir.dt.int16, tag="ei2_t")
            nc.scalar.dma_start(ei2_t[:], eidx2d)
            ed = ed_t[:]
            ei2 = ei2_t[:]
            t_lo = ed[:, 0:T]
            t_hi = ed[:, T:2 * T]
            pp = ed[:, 2 * T:3 * T]
            w = ed[:, 3 * T:4 * T]
            # constants (gpsimd queue is stalled behind the lib-code DMA,
            # but these finish well before their consumers)
            wz = wpool.tile([128, 1], F32, tag="wz")
            nc.gpsimd.memset(wz[:], 0.5)
            wb = wpool.tile([128, 1], F32, tag="wb")
            nc.gpsimd.memset(wb[:], 0.0)
            bias1 = wpool.tile([128, 1], F32, tag="bias1")
            nc.gpsimd.memset(bias1[:], 1.0 + 1e-6)
            ones_d = wpool.tile([128, T], F16, tag="ones_d")
            nc.gpsimd.memset(ones_d[:], 1.0)
            # Ln ACT table prewarm
            wo = wpool.tile([128, 1], F32, tag="wo")
            nc.scalar.activation(wo[:], wz[:], ACT.Ln, bias=wb[:])

            # iota on the DVE: prefix scan of ones, initial=-1 -> 0..127
            ones128 = wpool.tile([128, 128], F32, tag="ones128")
            nc.vector.memset(ones128[:], 1.0)
            iot = wpool.tile([128, 128], F32, tag="iot")
            nc.vector.tensor_tensor_scan(
                iot[:], ones128[:], ones128[:], -1.0,
                op0=ALU.add, op1=ALU.bypass,
            )
            io128 = iot[:]
            io32 = iot[:, 0:HI]

            # ---- value prep on Scalar: V = [logmsg | p] f32, D2 = p f16
            V = wpool.tile([128, 2 * T], F32, tag="V")
            nc.scalar.activation(V[:, 0:T], pp, ACT.Ln, scale=-1.0, bias=bias1[:])
            nc.scalar.copy(V[:, T:2 * T], pp)
            D2 = wpool.tile([128, T], F16, tag="D2")
            nc.scalar.copy(D2[:], pp)

            # ---- Vector: tgt hi one-hot, tgt lo one-hot, RS
            H_tgt = wpool.tile([128, T * HI], F16, tag="H_tgt")
            nc.vector.tensor_tensor(
                H_tgt[:].rearrange("p (t h) -> p t h", h=HI),
                io32.rearrange("p (o h) -> p o h", o=1).to_broadcast((128, T, HI)),
                t_hi.rearrange("p (t o) -> p t o", o=1).to_broadcast((128, T, HI)),
                op=ALU.is_equal,
            )
            A_tgt = wpool.tile([128, T * LO], F16, tag="A_tgt")
            nc.vector.tensor_tensor(
                A_tgt[:].rearrange("p (t l) -> p t l", l=LO),
                io128.rearrange("p (o l) -> p o l", o=1).to_broadcast((128, T, LO)),
                t_lo.rearrange("p (t o) -> p t o", o=1).to_broadcast((128, T, LO)),
                op=ALU.is_equal,
            )
            # ---- src side on GpSimd via local_scatter (-1 idx = dead)
            A_src = wpool.tile([128, T * LO], F16, tag="A_src")
            nc.gpsimd.local_scatter(
                A_src[:], ones_d[:], ei2[:, 0:T],
                channels=128, num_elems=T * LO, num_idxs=T,
            )
            # RS_all: per tile i the contiguous [rp_i(32) | rst_i(32)]
            RS_all = wpool.tile([128, T * 64], F16, tag="RS_all")
            nc.vector.tensor_tensor(
                RS_all[:].rearrange("p (t o h) -> p o t h", o=2, h=HI),
                H_tgt[:].rearrange("p (o t h) -> p o t h", o=1, h=HI)
                    .to_broadcast((128, 2, T, HI)),
                V[:].rearrange("p (o t) -> p o t", o=2)
                    .rearrange("p o (t h) -> p o t h", h=1)
                    .to_broadcast((128, 2, T, HI)),
                op=ALU.mult,
            )
            # ---- rsu on GpSimd via local_scatter of p-f16 at t*32+u_hi
            rsu_all = wpool.tile([128, T * HI], F16, tag="rsu_all")
            nc.gpsimd.local_scatter(
                rsu_all[:], D2[:], ei2[:, T:2 * T],
                channels=128, num_elems=T * HI, num_idxs=T,
            )
            # dp2 = sum p^2 w  (w = 2 - m from the host, 0 on pad slots)
            pw = wpool.tile([128, T], F32, tag="pw")
            nc.vector.tensor_tensor(pw[:], pp, w, op=ALU.mult)
            dp2scr = wpool.tile([128, T], F32, tag="dp2scr")
            dp2r = wpool.tile([128, 1], F32, tag="dp2r")
            nc.vector.scalar_tensor_tensor(
                dp2scr[:], pp, 1.0, pw[:],
                op0=ALU.mult, op1=ALU.mult, accum_out=dp2r[:],
            )

            # ---- scatter-add matmuls: P12 = [log_score(32) | s(32)]
            P12 = ppool.tile([128, 64], F32, tag="P12")
            for i in range(T):
                nc.tensor.matmul(
                    P12[:, 0:64],
                    A_tgt[:, i * LO:(i + 1) * LO],
                    RS_all[:, i * 64:(i + 1) * 64],
                    start=(i == 0), stop=False, skip_group_check=True,
                )
            for i in range(T):
                nc.tensor.matmul(
                    P12[:, 32:64],
                    A_src[:, i * LO:(i + 1) * LO],
                    rsu_all[:, i * HI:(i + 1) * HI],
                    start=False, stop=(i == T - 1), skip_group_check=True,
                )

            nc.vector.tensor_copy(C[:, 0:64], P12[:])
            nc.gpsimd.tensor_copy(C[:, 64:65], dp2r[:])

    # output DMA outside the TileContext: nothing waits on its completion
    # semaphore, so the transfer overlaps the fixed NEFF-epilogue tail
    nc.sync.dma_start(partiald, C).then_inc(odma_sem, 16)

    nc.compile()
    return nc


def _build_phase2():
    """Combine 8 partials -> final scalar. Runs on one core."""
    nc = bacc.Bacc("TRN2", target_bir_lowering=False, debug=False, num_devices=1)

    # partials, c innermost: partsa = x 0:32 (log_score), partsb = x 32:65
    # (s | dp2) then 64 cols whose row 0 holds batch[-64:] (batch is sorted
    # by construction, so max(batch) = max of that tail; values < 32 are
    # exact in f16).  Both on HWDGE queues (SP + Activation).
    partsad = nc.dram_tensor("partsa", [128, 256], F16, kind="ExternalInput").ap()
    partsbd = nc.dram_tensor("partsb", [128, 328], F16, kind="ExternalInput").ap()
    outd = nc.dram_tensor("out", [1, 1], F32, kind="ExternalOutput").ap()

    res = nc.alloc_sbuf_tensor("res_out", [1, 1], F32).ap()
    odma_sem = nc.alloc_semaphore("odma_sem")

    with tile.TileContext(nc) as tc:
        with (
            tc.tile_pool(name="pool", bufs=1) as pool,
            tc.tile_pool(name="psum", bufs=1, space="PSUM") as ppool,
        ):
            pta_t = pool.tile([128, 256], F16, tag="pta_t")
            nc.sync.dma_start(pta_t[:], partsad)
            ptb_t = pool.tile([128, 328], F16, tag="ptb_t")
            nc.scalar.dma_start(ptb_t[:], partsbd)
            pta = pta_t[:]
            ptb = ptb_t[:]
            # Exp table prewarm, inputs from gpsimd
            wz = pool.tile([128, 1], F32, tag="wz")
            nc.gpsimd.memset(wz[:], 0.5)
            wb = pool.tile([128, 1], F32, tag="wb")
            nc.gpsimd.memset(wb[:], 0.0)
            ones_t = pool.tile([128, 1], F16, tag="ones_t")
            nc.gpsimd.memset(ones_t[:], 1.0)
            wo = pool.tile([128, 1], F32, tag="wo")
            nc.scalar.activation(wo[:], wz[:], ACT.Exp, bias=wb[:])

            # 8-way partial sums on Vector; exp half first
            C2a = pool.tile([128, 32], F32, tag="C2a")
            nc.vector.tensor_reduce(
                C2a[:], pta.rearrange("p (x c) -> p x c", c=8),
                axis=AX.X, op=ALU.add,
            )
            C2b = pool.tile([128, 33], F32, tag="C2b")
            nc.vector.tensor_reduce(
                C2b[:], ptb[:, 0:264].rearrange("p (x c) -> p x c", c=8),
                axis=AX.X, op=ALU.add,
            )

            # f16 R keeps the ones-matmul single-pass (fp32 PE needs two
            # LDWEIGHTS passes); values are O(1e3), f16 rel err ~5e-4 ok
            R = pool.tile([128, 3], F16, tag="R")
            scr1 = pool.tile([128, HI], F32, tag="scr1")
            scr2 = pool.tile([128, HI], F32, tag="scr2")
            with nc.allow_low_precision("f16 partial row-sums, 5e-4 ok"):
                nc.scalar.activation(scr1[:], C2a[:], ACT.Exp, bias=wb[:],
                                     accum_out=R[:, 0:1])
                nc.vector.scalar_tensor_tensor(
                    scr2[:], C2b[:, 0:32], 1.0, C2b[:, 0:32],
                    op0=ALU.mult, op1=ALU.mult, accum_out=R[:, 1:2],
                )
            nc.gpsimd.tensor_copy(R[:, 2:3], C2b[:, 32:33])

            # num_graphs: rng = 100 / (max(batch) + 1); Vector is idle
            # here while Scalar finishes the exp accumulation
            ng = pool.tile([1, 1], F32, tag="ng")
            nc.vector.tensor_reduce(ng[:], ptb[0:1, 264:328], axis=AX.X, op=ALU.max)
            ng1 = pool.tile([1, 1], F32, tag="ng1")
            nc.vector.tensor_scalar(ng1[:], ng[:], 1.0, 0.01, op0=ALU.add, op1=ALU.mult)
            rng = pool.tile([1, 1], F32, tag="rng")
            nc.vector.reciprocal(rng[:], ng1[:])

            F = ppool.tile([1, 3], F32, tag="F")
            nc.tensor.matmul(F[:], ones_t[:], R[:], start=True, stop=True)

            l2 = pool.tile([1, 1], F32, tag="l2")
            SC = PENALTY_SCALE / N_NODES
            nc.scalar.activation(l2[:], F[:, 0:1], ACT.Copy,
                                 bias=-float(PAD_NODES) * SC, scale=SC)
            Fs = pool.tile([1, 2], F32, tag="Fs")
            nc.vector.tensor_copy(Fs[:], F[:, 1:3])
            d32 = pool.tile([1, 1], F32, tag="d32")
            nc.vector.tensor_tensor(d32[:], Fs[:, 0:1], Fs[:, 1:2], op=ALU.subtract)
            # res = d32 * (100/ng) + l2 in one fused op (scalar is an AP)
            nc.vector.scalar_tensor_tensor(
                res, d32[:], rng[:], l2[:], op0=ALU.mult, op1=ALU.add
            )

    # post-TileContext output DMA overlaps the fixed epilogue tail
    nc.sync.dma_start(outd, res).then_inc(odma_sem, 16)

    nc.compile()
    return nc


def _pack_core(tt, uu, p, T):
    """Pack one core's edge shard: f32 payload [128, 4*T] plus int16
    local_scatter index columns [128, 2*T]."""
    ne = tt.shape[0]
    npad = T * 128

    def pad(a, fill):
        out = np.full(npad, fill, np.float64)
        out[:ne] = a
        return out.reshape(T, 128).T  # [128, T]

    self_loop = uu == tt
    tvec = np.arange(T, dtype=np.float64)[None, :]
    t_lo = pad(tt % 128, 0.0)
    t_hi = pad(tt // 128, float(HI))     # sentinel hi -> matches nothing
    pf = pad(p, 0.0)
    wf = pad(2.0 - self_loop, 0.0)       # d_e = 2 - m_e, 0 on pad slots
    ed = np.concatenate([t_lo, t_hi, pf, wf], axis=1).astype(np.float32)
    # local_scatter indices: -1 rows (self-loops, pads) stay zero
    u_lo = pad(uu % 128, 0.0)
    u_hi = pad(uu // 128, 0.0)
    dead = pad(np.where(self_loop, 1.0, 0.0), 1.0) > 0.5
    i_src = np.where(dead, -1.0, tvec * LO + u_lo)
    i_rsu = np.where(dead, -1.0, tvec * HI + u_hi)
    ei2 = np.concatenate([i_src, i_rsu], axis=1).astype(np.int16)
    return ed, ei2


_CACHE = {}


def _get(name, builder, *a):
    if name not in _CACHE:
        _CACHE[name] = builder(*a)
    return _CACHE[name]


def kernel(x, edge_index, edge_feature, batch, _trace=False):
    x = np.asarray(x)
    ei = np.asarray(edge_index).astype(np.int64)
    p = np.asarray(edge_feature).astype(np.float32)[:, 0]
    batch = np.asarray(batch).astype(np.int64)

    uu_all = ei[0].astype(np.float64)
    tt_all = ei[1].astype(np.float64)

    # ---- phase 1: per-core partials (no cross-core dependencies)
    nc1 = _get("p1", _build_phase1, TPC)
    in_maps = []
    for c in range(N_CORES):
        sl = slice(c * EPC, (c + 1) * EPC)
        ed, ei2 = _pack_core(tt_all[sl], uu_all[sl], p[sl], TPC)
        in_maps.append({"edata": ed, "eidx2": ei2})
    r1 = bass_utils.run_bass_kernel_spmd(
        nc1, in_maps, core_ids=list(range(N_CORES)), trace=_trace
    )

    # gather/unshard the per-core partials (pure data movement)
    parts = np.stack(
        [np.asarray(r1.results[c]["partial"]) for c in range(N_CORES)], axis=2
    ).astype(np.float16)                               # [p, x, c], c innermost

    # ---- phase 2: combine on one core
    nc2 = _get("p2", _build_phase2)
    btail = np.zeros((128, 64), np.float16)
    btail[0, :] = batch[-64:].astype(np.float16)
    partsa = parts[:, 0:32, :].reshape(128, 256)
    partsb = np.concatenate([parts[:, 32:65, :].reshape(128, 264), btail], axis=1)
    r2 = bass_utils.run_bass_kernel_spmd(
        nc2, [{"partsa": partsa, "partsb": partsb}], core_ids=[0], trace=_trace,
    )
    out = np.asarray(r2.results[0]["out"], dtype=np.float32).reshape(1, 1)
    if _trace:
        kernel.last_results = (r1, r2)
    return out


# revision 39
# speedup vs baseline: 1.0045x; 1.0045x over previous
"""Trainium2 Bass kernel for nn_ErdosLoss (graph loss function).

Math (reference reformulated, validated to ~1e-6 rel err):
  penalty:  log_score = scatter_add(log(1 - p + 1e-6), tgt)   over N nodes
            loss2 = mean(exp(log_score)) * 9600
  loss3:    p @ triu(H H^T, 1) @ p^T  ==  (||s||^2 - sum_e d_e p_e^2) / 2
            where s = scatter_add(p, tgt) + scatter_add(p, src),
            self-loop edges get -1 local_scatter indices (host-side index
            prep) so their src contribution vanishes; d_e = 2 - m_e is
            shipped as an index-derived weight column w.
  out = loss2 + 200 * loss3 / num_graphs,  num_graphs = max(batch) + 1.

Device strategy (8 NeuronCores, SPMD, two launches, no collectives):
  Per-NEFF measured window = first named inst -> trace end, and the trace
  end includes a fixed ~7.4us framework tail (the NEFF execution-loop
  preamble: each engine clears its ~51-semaphore block one inst at a
  time).  Per-launch serial anatomy: input DMA latency (trigger 565ns +
  HWDGE 625ns + delay 650ns + transfer + sem prop), compute, output DMA,
  pool teardown, tail.  Design choices driven by that:
  - The OUTPUT DMA is issued after the TileContext closes (raw staging
    buffer + explicit completion sem nobody waits on), so no pool barrier
    waits on it: the ~2us transfer overlaps the fixed tail.  (Issuing the
    INPUT DMAs pre-TileContext with manual sem waits was tried and hangs
    the NEFF on hardware — keep input DMAs tile-tracked.)
  - Phase 1 (8 cores, edge-sharded 750/core): scatter-add via one-hot
    matmul with node = 128*hi + lo decomposition (N padded to 4096).
    iota comes from a DVE prefix-scan of ones (no GPSIMD standard-lib
    dependency); the local_scatter Q7 library is swapped in as the first
    gpsimd op so its ~2.2us code DMA overlaps the input DMA.  Engine
    split:
    Vector builds H_tgt + A_tgt one-hots + RS (= H x [logmsg | p]) and
    the dp2 row-sum; GpSimd builds the src side via local_scatter (a 1
    at t*128+u_lo; p-f16 at t*32+u_hi), -1 indices killing self-loops
    and pads.  TensorE contracts into PSUM [128lo, 64] (= log_score | s).
  - Host gathers the 8 partials (pure data movement, c-innermost).
  - Phase 2 (1 core): 8-way reduces + exp/square row-sums (accum_out),
    f16 ones-matmul partition reduce, num_graphs = max(batch)+1 from the
    sorted batch tail, fused scalar chain.
  Engine-queue FIFO order is load-bearing: ops are emitted in
  critical-path order per engine.
"""

import numpy as np

import concourse.bacc as bacc
import concourse.mybir as mybir
import concourse.tile as tile
from concourse import bass_utils
from concourse import library_config

F32 = mybir.dt.float32
F16 = mybir.dt.float16
ALU = mybir.AluOpType
ACT = mybir.ActivationFunctionType
AX = mybir.AxisListType

N_NODES = 4000
N_EDGES = 6000
N_CORES = 8
N_PAD = 4096          # 128 * 32
HI = 32               # node hi-digits
LO = 128              # node lo-digits
PENALTY_SCALE = 16 * 200 * 3   # 9600
PAD_NODES = N_PAD - N_NODES    # 96 padded nodes, each contributes exp(0)=1

EPC = N_EDGES // N_CORES       # 750 edges per core
TPC = (EPC + 127) // 128       # 6 edge tiles per core


def _build_phase1(T: int):
    """Per-core partial computation: out 'partial' [128, 65] f16."""
    nc = bacc.Bacc("TRN2", target_bir_lowering=False, debug=False, num_devices=1)

    NC = 4 * T
    edatad = nc.dram_tensor("edata", [128, NC], F32, kind="ExternalInput").ap()
    eidx2d = nc.dram_tensor("eidx2", [128, 2 * T], mybir.dt.int16,
                            kind="ExternalInput").ap()
    partiald = nc.dram_tensor("partial", [128, 65], F16, kind="ExternalOutput").ap()

    # raw staging buffer for the post-TileContext output DMA
    C = nc.alloc_sbuf_tensor("C_out", [128, 65], F16).ap()
    odma_sem = nc.alloc_semaphore("odma_sem")

    with tile.TileContext(nc) as tc:
        with (
            tc.tile_pool(name="work", bufs=1) as wpool,
            tc.tile_pool(name="psum", bufs=1, space="PSUM") as ppool,
        ):
            ed_t = wpool.tile([128, NC], F32, tag="ed_t")
            nc.sync.dma_start(ed_t[:], edatad)
            ei2_t = wpool.tile([128, 2 * T], mybir.dt.int16, tag="ei2_t")
            nc.scalar.dma_start(ei2_t[:], eidx2d)
            ed = ed_t[:]
            ei2 = ei2_t[:]
            t_lo = ed[:, 0:T]
            t_hi = ed[:, T:2 * T]
            pp = ed[:, 2 * T:3 * T]
            w = ed[:, 3 * T:4 * T]
            # constants (gpsimd queue is stalled behind the lib-code DMA,
            # but these finish well before their consumers)
            wz = wpool.tile([128, 1], F32, tag="wz")
            nc.gpsimd.memset(wz[:], 0.5)
            wb = wpool.tile([128, 1], F32, tag="wb")
            nc.gpsimd.memset(wb[:], 0.0)
            bias1 = wpool.tile([128, 1], F32, tag="bias1")
            nc.gpsimd.memset(bias1[:], 1.0 + 1e-6)
            ones_d = wpool.tile([128, T], F16, tag="ones_d")
            nc.gpsimd.memset(ones_d[:], 1.0)
            # Ln ACT table prewarm
            wo = wpool.tile([128, 1], F32, tag="wo")
            nc.scalar.activation(wo[:], wz[:], ACT.Ln, bias=wb[:])

            # iota on the DVE: prefix scan of ones, initial=-1 -> 0..127
            ones128 = wpool.tile([128, 128], F32, tag="ones128")
            nc.vector.memset(ones128[:], 1.0)
            iot = wpool.tile([128, 128], F32, tag="iot")
            nc.vector.tensor_tensor_scan(
                iot[:], ones128[:], ones128[:], -1.0,
                op0=ALU.add, op1=ALU.bypass,
            )
            io128 = iot[:]
            io32 = iot[:, 0:HI]

            # ---- value prep on Scalar: V = [logmsg | p] f32, D2 = p f16
            V = wpool.tile([128, 2 * T], F32, tag="V")
            nc.scalar.activation(V[:, 0:T], pp, ACT.Ln, scale=-1.0, bias=bias1[:])
            nc.scalar.copy(V[:, T:2 * T], pp)
            D2 = wpool.tile([128, T], F16, tag="D2")
            nc.scalar.copy(D2[:], pp)

            # ---- Vector: tgt hi one-hot, tgt lo one-hot, RS
            H_tgt = wpool.tile([128, T * HI], F16, tag="H_tgt")
            nc.vector.tensor_tensor(
                H_tgt[:].rearrange("p (t h) -> p t h", h=HI),
                io32.rearrange("p (o h) -> p o h", o=1).to_broadcast((128, T, HI)),
                t_hi.rearrange("p (t o) -> p t o", o=1).to_broadcast((128, T, HI)),
                op=ALU.is_equal,
            )
            A_tgt = wpool.tile([128, T * LO], F16, tag="A_tgt")
            nc.vector.tensor_tensor(
                A_tgt[:].rearrange("p (t l) -> p t l", l=LO),
                io128.rearrange("p (o l) -> p o l", o=1).to_broadcast((128, T, LO)),
                t_lo.rearrange("p (t o) -> p t o", o=1).to_broadcast((128, T, LO)),
                op=ALU.is_equal,
            )
            # ---- src side on GpSimd via local_scatter (-1 idx = dead).
            # The Q7 library swap is emitted here, after the Scalar
            # activations, so the ACT pass's table-restore lands off the
            # Ln critical path; in the gpsimd queue it still precedes the
            # local_scatters and its code DMA overlaps the input DMA.
            nc.gpsimd.load_library(library_config.local_scatter)
            A_src = wpool.tile([128, T * LO], F16, tag="A_src")
            nc.gpsimd.local_scatter(
                A_src[:], ones_d[:], ei2[:, 0:T],
                channels=128, num_elems=T * LO, num_idxs=T,
            )
            # RS_all: per tile i the contiguous [rp_i(32) | rst_i(32)]
            RS_all = wpool.tile([128, T * 64], F16, tag="RS_all")
            nc.vector.tensor_tensor(
                RS_all[:].rearrange("p (t o h) -> p o t h", o=2, h=HI),
                H_tgt[:].rearrange("p (o t h) -> p o t h", o=1, h=HI)
                    .to_broadcast((128, 2, T, HI)),
                V[:].rearrange("p (o t) -> p o t", o=2)
                    .rearrange("p o (t h) -> p o t h", h=1)
                    .to_broadcast((128, 2, T, HI)),
                op=ALU.mult,
            )
            # ---- rsu on GpSimd via local_scatter of p-f16 at t*32+u_hi
            rsu_all = wpool.tile([128, T * HI], F16, tag="rsu_all")
            nc.gpsimd.local_scatter(
                rsu_all[:], D2[:], ei2[:, T:2 * T],
                channels=128, num_elems=T * HI, num_idxs=T,
            )
            # dp2 = sum p^2 w  (w = 2 - m from the host, 0 on pad slots)
            pw = wpool.tile([128, T], F32, tag="pw")
            nc.vector.tensor_tensor(pw[:], pp, w, op=ALU.mult)
            dp2scr = wpool.tile([128, T], F32, tag="dp2scr")
            dp2r = wpool.tile([128, 1], F32, tag="dp2r")
            nc.vector.scalar_tensor_tensor(
                dp2scr[:], pp, 1.0, pw[:],
                op0=ALU.mult, op1=ALU.mult, accum_out=dp2r[:],
            )

            # ---- scatter-add matmuls: P12 = [log_score(32) | s(32)]
            P12 = ppool.tile([128, 64], F32, tag="P12")
            for i in range(T):
                nc.tensor.matmul(
                    P12[:, 0:64],
                    A_tgt[:, i * LO:(i + 1) * LO],
                    RS_all[:, i * 64:(i + 1) * 64],
                    start=(i == 0), stop=False, skip_group_check=True,
                )
            for i in range(T):
                nc.tensor.matmul(
                    P12[:, 32:64],
                    A_src[:, i * LO:(i + 1) * LO],
                    rsu_all[:, i * HI:(i + 1) * HI],
                    start=False, stop=(i == T - 1), skip_group_check=True,
                )

            nc.vector.tensor_copy(C[:, 0:64], P12[:])
            nc.gpsimd.tensor_copy(C[:, 64:65], dp2r[:])

    # output DMA outside the TileContext: nothing waits on its completion
    # semaphore, so the transfer overlaps the fixed NEFF-epilogue tail
    nc.sync.dma_start(partiald, C).then_inc(odma_sem, 16)

    nc.compile()
    return nc


def _build_phase2():
    """Combine 8 partials -> final scalar. Runs on one core."""
    nc = bacc.Bacc("TRN2", target_bir_lowering=False, debug=False, num_devices=1)

    # partials, c innermost: partsa = x 0:32 (log_score), partsb = x 32:65
    # (s | dp2) then 64 cols whose row 0 holds batch[-64:] (batch is sorted
    # by construction, so max(batch) = max of that tail; values < 32 are
    # exact in f16).  Both on HWDGE queues (SP + Activation).
    partsad = nc.dram_tensor("partsa", [128, 256], F16, kind="ExternalInput").ap()
    partsbd = nc.dram_tensor("partsb", [128, 328], F16, kind="ExternalInput").ap()
    outd = nc.dram_tensor("out", [1, 1], F32, kind="ExternalOutput").ap()

    res = nc.alloc_sbuf_tensor("res_out", [1, 1], F32).ap()
    odma_sem = nc.alloc_semaphore("odma_sem")

    with tile.TileContext(nc) as tc:
        with (
            tc.tile_pool(name="pool", bufs=1) as pool,
            tc.tile_pool(name="psum", bufs=1, space="PSUM") as ppool,
        ):
            pta_t = pool.tile([128, 256], F16, tag="pta_t")
            nc.sync.dma_start(pta_t[:], partsad)
            ptb_t = pool.tile([128, 328], F16, tag="ptb_t")
            nc.scalar.dma_start(ptb_t[:], partsbd)
            pta = pta_t[:]
            ptb = ptb_t[:]
            # Exp table prewarm, inputs from gpsimd
            wz = pool.tile([128, 1], F32, tag="wz")
            nc.gpsimd.memset(wz[:], 0.5)
            wb = pool.tile([128, 1], F32, tag="wb")
            nc.gpsimd.memset(wb[:], 0.0)
            ones_t = pool.tile([128, 1], F16, tag="ones_t")
            nc.gpsimd.memset(ones_t[:], 1.0)
            wo = pool.tile([128, 1], F32, tag="wo")
            nc.scalar.activation(wo[:], wz[:], ACT.Exp, bias=wb[:])

            # 8-way partial sums on Vector; exp half first
            C2a = pool.tile([128, 32], F32, tag="C2a")
            nc.vector.tensor_reduce(
                C2a[:], pta.rearrange("p (x c) -> p x c", c=8),
                axis=AX.X, op=ALU.add,
            )
            C2b = pool.tile([128, 33], F32, tag="C2b")
            nc.vector.tensor_reduce(
                C2b[:], ptb[:, 0:264].rearrange("p (x c) -> p x c", c=8),
                axis=AX.X, op=ALU.add,
            )

            # f16 R keeps the ones-matmul single-pass (fp32 PE needs two
            # LDWEIGHTS passes); values are O(1e3), f16 rel err ~5e-4 ok.
            # R[:,1] = (sum s^2) - dp2 per partition, so the final scalar
            # chain is just res = F1*rng + l2 with F read from PSUM.
            R = pool.tile([128, 2], F16, tag="R")
            scr1 = pool.tile([128, HI], F32, tag="scr1")
            scr2 = pool.tile([128, HI], F32, tag="scr2")
            r1s = pool.tile([128, 1], F32, tag="r1s")
            with nc.allow_low_precision("f16 partial row-sums, 5e-4 ok"):
                nc.scalar.activation(scr1[:], C2a[:], ACT.Exp, bias=wb[:],
                                     accum_out=R[:, 0:1])
                nc.vector.scalar_tensor_tensor(
                    scr2[:], C2b[:, 0:32], 1.0, C2b[:, 0:32],
                    op0=ALU.mult, op1=ALU.mult, accum_out=r1s[:],
                )
                nc.vector.tensor_tensor(R[:, 1:2], r1s[:], C2b[:, 32:33],
                                        op=ALU.subtract)

            # num_graphs: rng = 100 / (max(batch) + 1); Vector is idle
            # here while Scalar finishes the exp accumulation
            ng = pool.tile([1, 1], F32, tag="ng")
            nc.vector.tensor_reduce(ng[:], ptb[0:1, 264:328], axis=AX.X, op=ALU.max)
            ng1 = pool.tile([1, 1], F32, tag="ng1")
            nc.vector.tensor_scalar(ng1[:], ng[:], 1.0, 0.01, op0=ALU.add, op1=ALU.mult)
            rng = pool.tile([1, 1], F32, tag="rng")
            nc.vector.reciprocal(rng[:], ng1[:])

            F = ppool.tile([1, 2], F32, tag="F")
            nc.tensor.matmul(F[:], ones_t[:], R[:], start=True, stop=True)

            l2 = pool.tile([1, 1], F32, tag="l2")
            SC = PENALTY_SCALE / N_NODES
            nc.scalar.activation(l2[:], F[:, 0:1], ACT.Copy,
                                 bias=-float(PAD_NODES) * SC, scale=SC)
            # res = F1 * (100/ng) + l2 — F1 read straight from PSUM
            nc.vector.scalar_tensor_tensor(
                res, F[:, 1:2], rng[:], l2[:], op0=ALU.mult, op1=ALU.add
            )

    # post-TileContext output DMA overlaps the fixed epilogue tail
    nc.sync.dma_start(outd, res).then_inc(odma_sem, 16)

    nc.compile()
    return nc


def _pack_core(tt, uu, p, T):
    """Pack one core's edge shard: f32 payload [128, 4*T] plus int16
    local_scatter index columns [128, 2*T]."""
    ne = tt.shape[0]
    npad = T * 128

    def pad(a, fill):
        out = np.full(npad, fill, np.float64)
        out[:ne] = a
        return out.reshape(T, 128).T  # [128, T]

    self_loop = uu == tt
    tvec = np.arange(T, dtype=np.float64)[None, :]
    t_lo = pad(tt % 128, 0.0)
    t_hi = pad(tt // 128, float(HI))     # sentinel hi -> matches nothing
    pf = pad(p, 0.0)
    wf = pad(2.0 - self_loop, 0.0)       # d_e = 2 - m_e, 0 on pad slots
    ed = np.concatenate([t_lo, t_hi, pf, wf], axis=1).astype(np.float32)
    # local_scatter indices: -1 rows (self-loops, pads) stay zero
    u_lo = pad(uu % 128, 0.0)
    u_hi = pad(uu // 128, 0.0)
    dead = pad(np.where(self_loop, 1.0, 0.0), 1.0) > 0.5
    i_src = np.where(dead, -1.0, tvec * LO + u_lo)
    i_rsu = np.where(dead, -1.0, tvec * HI + u_hi)
    ei2 = np.concatenate([i_src, i_rsu], axis=1).astype(np.int16)
    return ed, ei2


_CACHE = {}


def _get(name, builder, *a):
    if name not in _CACHE:
        _CACHE[name] = builder(*a)
    return _CACHE[name]


def kernel(x, edge_index, edge_feature, batch, _trace=False):
    x = np.asarray(x)
    ei = np.asarray(edge_index).astype(np.int64)
    p = np.asarray(edge_feature).astype(np.float32)[:, 0]
    batch = np.asarray(batch).astype(np.int64)

    uu_all = ei[0].astype(np.float64)
    tt_all = ei[1].astype(np.float64)

    # ---- phase 1: per-core partials (no cross-core dependencies)
    nc1 = _get("p1", _build_phase1, TPC)
    in_maps = []
    for c in range(N_CORES):
        sl = slice(c * EPC, (c + 1) * EPC)
        ed, ei2 = _pack_core(tt_all[sl], uu_all[sl], p[sl], TPC)
        in_maps.append({"edata": ed, "eidx2": ei2})
    r1 = bass_utils.run_bass_kernel_spmd(
        nc1, in_maps, core_ids=list(range(N_CORES)), trace=_trace
    )

    # gather/unshard the per-core partials (pure data movement)
    parts = np.stack(
        [np.asarray(r1.results[c]["partial"]) for c in range(N_CORES)], axis=2
    ).astype(np.float16)                               # [p, x, c], c innermost

    # ---- phase 2: combine on one core
    nc2 = _get("p2", _build_phase2)
    btail = np.zeros((128, 64), np.float16)
    btail[0, :] = batch[-64:].astype(np.float16)
    partsa = parts[:, 0:32, :].reshape(128, 256)
    partsb = np.concatenate([parts[:, 32:65, :].reshape(128, 264), btail], axis=1)
    r2 = bass_utils.run_bass_kernel_spmd(
        nc2, [{"partsa": partsa, "partsb": partsb}], core_ids=[0], trace=_trace,
    )
    out = np.asarray(r2.results[0]["out"], dtype=np.float32).reshape(1, 1)
    if _trace:
        kernel.last_results = (r1, r2)
    return out


# revision 40
# speedup vs baseline: 1.0199x; 1.0153x over previous
"""Trainium2 Bass kernel for nn_ErdosLoss (graph loss function).

Math (reference reformulated, validated to ~1e-6 rel err):
  penalty:  log_score = scatter_add(log(1 - p + 1e-6), tgt)   over N nodes
            loss2 = mean(exp(log_score)) * 9600
  loss3:    p @ triu(H H^T, 1) @ p^T  ==  (||s||^2 - sum_e d_e p_e^2) / 2
            where s = scatter_add(p, tgt) + scatter_add(p, src),
            self-loop edges get -1 local_scatter indices (host-side index
            prep) so their src contribution vanishes; d_e = 2 - m_e is
            shipped as an index-derived weight column w.
  out = loss2 + 200 * loss3 / num_graphs,  num_graphs = max(batch) + 1.

Device strategy (8 NeuronCores, SPMD, two launches, no collectives):
  Per-NEFF measured window = first named inst -> trace end, and the trace
  end includes a fixed ~7.4us framework tail (the NEFF execution-loop
  preamble: each engine clears its ~51-semaphore block one inst at a
  time).  Per-launch serial anatomy: input DMA latency (trigger 565ns +
  HWDGE 625ns + delay 650ns + transfer + sem prop), compute, output DMA,
  pool teardown, tail.  Design choices driven by that:
  - The OUTPUT DMA is issued after the TileContext closes (raw staging
    buffer + explicit completion sem nobody waits on), so no pool barrier
    waits on it: the ~2us transfer overlaps the fixed tail.  (Issuing the
    INPUT DMAs pre-TileContext with manual sem waits was tried and hangs
    the NEFF on hardware — keep input DMAs tile-tracked.)
  - Phase 1 (8 cores, edge-sharded 750/core): scatter-add via one-hot
    matmul with node = 128*hi + lo decomposition (N padded to 4096).
    iota comes from a DVE prefix-scan of ones (no GPSIMD standard-lib
    dependency); the local_scatter Q7 library is swapped in as the first
    gpsimd op so its ~2.2us code DMA overlaps the input DMA.  Engine
    split:
    Vector builds H_tgt + A_tgt one-hots + RS (= H x [logmsg | p]) and
    the dp2 row-sum; GpSimd builds the src side via local_scatter (a 1
    at t*128+u_lo; p-f16 at t*32+u_hi), -1 indices killing self-loops
    and pads.  TensorE contracts into PSUM [128lo, 64] (= log_score | s).
  - Host gathers the 8 partials (pure data movement, c-innermost).
  - Phase 2 (1 core): 8-way reduces + exp/square row-sums (accum_out),
    f16 ones-matmul partition reduce, num_graphs = max(batch)+1 from the
    sorted batch tail, fused scalar chain.
  Engine-queue FIFO order is load-bearing: ops are emitted in
  critical-path order per engine.
"""

import numpy as np

import concourse.bacc as bacc
import concourse.mybir as mybir
import concourse.tile as tile
from concourse import bass_utils
from concourse import library_config

F32 = mybir.dt.float32
F16 = mybir.dt.float16
ALU = mybir.AluOpType
ACT = mybir.ActivationFunctionType
AX = mybir.AxisListType

N_NODES = 4000
N_EDGES = 6000
N_CORES = 8
N_PAD = 4096          # 128 * 32
HI = 32               # node hi-digits
LO = 128              # node lo-digits
PENALTY_SCALE = 16 * 200 * 3   # 9600
PAD_NODES = N_PAD - N_NODES    # 96 padded nodes, each contributes exp(0)=1

EPC = N_EDGES // N_CORES       # 750 edges per core
TPC = (EPC + 127) // 128       # 6 edge tiles per core


def _build_phase1(T: int):
    """Per-core partial computation: out 'partial' [128, 65] f16."""
    nc = bacc.Bacc("TRN2", target_bir_lowering=False, debug=False, num_devices=1)

    NC = 4 * T
    edatad = nc.dram_tensor("edata", [128, NC], F32, kind="ExternalInput").ap()
    eidx2d = nc.dram_tensor("eidx2", [128, 2 * T], mybir.dt.int16,
                            kind="ExternalInput").ap()
    partiald = nc.dram_tensor("partial", [128, 65], F16, kind="ExternalOutput").ap()

    # raw staging buffer for the post-TileContext output DMA
    C = nc.alloc_sbuf_tensor("C_out", [128, 65], F16).ap()
    odma_sem = nc.alloc_semaphore("odma_sem")

    with tile.TileContext(nc) as tc:
        with (
            tc.tile_pool(name="work", bufs=1) as wpool,
            tc.tile_pool(name="psum", bufs=1, space="PSUM") as ppool,
        ):
            ed_t = wpool.tile([128, NC], F32, tag="ed_t")
            nc.sync.dma_start(ed_t[:], edatad)
            ei2_t = wpool.tile([128, 2 * T], mybir.dt.int16, tag="ei2_t")
            nc.scalar.dma_start(ei2_t[:], eidx2d)
            ed = ed_t[:]
            ei2 = ei2_t[:]
            t_lo = ed[:, 0:T]
            t_hi = ed[:, T:2 * T]
            pp = ed[:, 2 * T:3 * T]
            w = ed[:, 3 * T:4 * T]
            # constants (gpsimd queue is stalled behind the lib-code DMA,
            # but these finish well before their consumers)
            wz = wpool.tile([128, 1], F32, tag="wz")
            nc.gpsimd.memset(wz[:], 0.5)
            wb = wpool.tile([128, 1], F32, tag="wb")
            nc.gpsimd.memset(wb[:], 0.0)
            bias1 = wpool.tile([128, 1], F32, tag="bias1")
            nc.gpsimd.memset(bias1[:], 1.0 + 1e-6)
            ones_d = wpool.tile([128, T], F16, tag="ones_d")
            nc.gpsimd.memset(ones_d[:], 1.0)
            # Ln ACT table prewarm
            wo = wpool.tile([128, 1], F32, tag="wo")
            nc.scalar.activation(wo[:], wz[:], ACT.Ln, bias=wb[:])

            # iota on the DVE: prefix scan of ones, initial=-1 -> 0..127
            ones128 = wpool.tile([128, 128], F32, tag="ones128")
            nc.vector.memset(ones128[:], 1.0)
            iot = wpool.tile([128, 128], F32, tag="iot")
            nc.vector.tensor_tensor_scan(
                iot[:], ones128[:], ones128[:], -1.0,
                op0=ALU.add, op1=ALU.bypass,
            )
            io128 = iot[:]
            io32 = iot[:, 0:HI]

            # ---- value prep on Scalar: V = [logmsg | p] f32, D2 = p f16
            V = wpool.tile([128, 2 * T], F32, tag="V")
            nc.scalar.activation(V[:, 0:T], pp, ACT.Ln, scale=-1.0, bias=bias1[:])
            nc.scalar.copy(V[:, T:2 * T], pp)
            D2 = wpool.tile([128, T], F16, tag="D2")
            nc.scalar.copy(D2[:], pp)

            # ---- Vector: tgt hi one-hot, tgt lo one-hot, RS
            H_tgt = wpool.tile([128, T * HI], F16, tag="H_tgt")
            nc.vector.tensor_tensor(
                H_tgt[:].rearrange("p (t h) -> p t h", h=HI),
                io32.rearrange("p (o h) -> p o h", o=1).to_broadcast((128, T, HI)),
                t_hi.rearrange("p (t o) -> p t o", o=1).to_broadcast((128, T, HI)),
                op=ALU.is_equal,
            )
            A_tgt = wpool.tile([128, T * LO], F16, tag="A_tgt")
            nc.vector.tensor_tensor(
                A_tgt[:].rearrange("p (t l) -> p t l", l=LO),
                io128.rearrange("p (o l) -> p o l", o=1).to_broadcast((128, T, LO)),
                t_lo.rearrange("p (t o) -> p t o", o=1).to_broadcast((128, T, LO)),
                op=ALU.is_equal,
            )
            # ---- src side on GpSimd via local_scatter (-1 idx = dead).
            # The Q7 library swap is emitted here, after the Scalar
            # activations, so the ACT pass's table-restore lands off the
            # Ln critical path; in the gpsimd queue it still precedes the
            # local_scatters and its code DMA overlaps the input DMA.
            nc.gpsimd.load_library(library_config.local_scatter)
            A_src = wpool.tile([128, T * LO], F16, tag="A_src")
            nc.gpsimd.local_scatter(
                A_src[:], ones_d[:], ei2[:, 0:T],
                channels=128, num_elems=T * LO, num_idxs=T,
            )
            # RS_all: per tile i the contiguous [rp_i(32) | rst_i(32)]
            RS_all = wpool.tile([128, T * 64], F16, tag="RS_all")
            nc.vector.tensor_tensor(
                RS_all[:].rearrange("p (t o h) -> p o t h", o=2, h=HI),
                H_tgt[:].rearrange("p (o t h) -> p o t h", o=1, h=HI)
                    .to_broadcast((128, 2, T, HI)),
                V[:].rearrange("p (o t) -> p o t", o=2)
                    .rearrange("p o (t h) -> p o t h", h=1)
                    .to_broadcast((128, 2, T, HI)),
                op=ALU.mult,
            )
            # ---- rsu on GpSimd via local_scatter of p-f16 at t*32+u_hi
            rsu_all = wpool.tile([128, T * HI], F16, tag="rsu_all")
            nc.gpsimd.local_scatter(
                rsu_all[:], D2[:], ei2[:, T:2 * T],
                channels=128, num_elems=T * HI, num_idxs=T,
            )
            # dp2 = sum p^2 w  (w = 2 - m from the host, 0 on pad slots)
            pw = wpool.tile([128, T], F32, tag="pw")
            nc.vector.tensor_tensor(pw[:], pp, w, op=ALU.mult)
            dp2scr = wpool.tile([128, T], F32, tag="dp2scr")
            dp2r = wpool.tile([128, 1], F32, tag="dp2r")
            nc.vector.scalar_tensor_tensor(
                dp2scr[:], pp, 1.0, pw[:],
                op0=ALU.mult, op1=ALU.mult, accum_out=dp2r[:],
            )

            # ---- scatter-add matmuls: P12 = [log_score(32) | s(32)]
            P12 = ppool.tile([128, 64], F32, tag="P12")
            for i in range(T):
                nc.tensor.matmul(
                    P12[:, 0:64],
                    A_tgt[:, i * LO:(i + 1) * LO],
                    RS_all[:, i * 64:(i + 1) * 64],
                    start=(i == 0), stop=False, skip_group_check=True,
                )
            for i in range(T):
                nc.tensor.matmul(
                    P12[:, 32:64],
                    A_src[:, i * LO:(i + 1) * LO],
                    rsu_all[:, i * HI:(i + 1) * HI],
                    start=False, stop=(i == T - 1), skip_group_check=True,
                )

            nc.vector.tensor_copy(C[:, 0:64], P12[:])
            nc.gpsimd.tensor_copy(C[:, 64:65], dp2r[:])

    # output DMA outside the TileContext: nothing waits on its completion
    # semaphore, so the transfer overlaps the fixed NEFF-epilogue tail;
    # the gpsimd SWDGE trigger costs ~25ns on-queue (vs 565-700ns HWDGE
    # config on SP), so the final all-engine barrier isn't delayed
    nc.gpsimd.dma_start(partiald, C).then_inc(odma_sem, 16)

    nc.compile()
    return nc


def _build_phase2():
    """Combine 8 partials -> final scalar. Runs on one core."""
    nc = bacc.Bacc("TRN2", target_bir_lowering=False, debug=False, num_devices=1)

    # partials, c innermost: partsa = x 0:32 (log_score), partsb = x 32:65
    # (s | dp2) then 64 cols whose row 0 holds batch[-64:] (batch is sorted
    # by construction, so max(batch) = max of that tail; values < 32 are
    # exact in f16).  Both on HWDGE queues (SP + Activation).
    partsad = nc.dram_tensor("partsa", [128, 256], F16, kind="ExternalInput").ap()
    partsbd = nc.dram_tensor("partsb", [128, 328], F16, kind="ExternalInput").ap()
    outd = nc.dram_tensor("out", [1, 1], F32, kind="ExternalOutput").ap()

    res = nc.alloc_sbuf_tensor("res_out", [1, 1], F32).ap()
    odma_sem = nc.alloc_semaphore("odma_sem")

    with tile.TileContext(nc) as tc:
        with (
            tc.tile_pool(name="pool", bufs=1) as pool,
            tc.tile_pool(name="psum", bufs=1, space="PSUM") as ppool,
        ):
            pta_t = pool.tile([128, 256], F16, tag="pta_t")
            nc.sync.dma_start(pta_t[:], partsad)
            ptb_t = pool.tile([128, 328], F16, tag="ptb_t")
            nc.scalar.dma_start(ptb_t[:], partsbd)
            pta = pta_t[:]
            ptb = ptb_t[:]
            # Exp table prewarm, inputs from gpsimd
            wz = pool.tile([128, 1], F32, tag="wz")
            nc.gpsimd.memset(wz[:], 0.5)
            wb = pool.tile([128, 1], F32, tag="wb")
            nc.gpsimd.memset(wb[:], 0.0)
            ones_t = pool.tile([128, 1], F16, tag="ones_t")
            nc.gpsimd.memset(ones_t[:], 1.0)
            wo = pool.tile([128, 1], F32, tag="wo")
            nc.scalar.activation(wo[:], wz[:], ACT.Exp, bias=wb[:])

            # 8-way partial sums on Vector; exp half first
            C2a = pool.tile([128, 32], F32, tag="C2a")
            nc.vector.tensor_reduce(
                C2a[:], pta.rearrange("p (x c) -> p x c", c=8),
                axis=AX.X, op=ALU.add,
            )
            C2b = pool.tile([128, 33], F32, tag="C2b")
            nc.vector.tensor_reduce(
                C2b[:], ptb[:, 0:264].rearrange("p (x c) -> p x c", c=8),
                axis=AX.X, op=ALU.add,
            )

            # f16 R keeps the ones-matmul single-pass (fp32 PE needs two
            # LDWEIGHTS passes); values are O(1e3), f16 rel err ~5e-4 ok.
            # R[:,1] = (sum s^2) - dp2 per partition, so the final scalar
            # chain is just res = F1*rng + l2 with F read from PSUM.
            R = pool.tile([128, 2], F16, tag="R")
            scr1 = pool.tile([128, HI], F32, tag="scr1")
            scr2 = pool.tile([128, HI], F32, tag="scr2")
            r1s = pool.tile([128, 1], F32, tag="r1s")
            with nc.allow_low_precision("f16 partial row-sums, 5e-4 ok"):
                nc.scalar.activation(scr1[:], C2a[:], ACT.Exp, bias=wb[:],
                                     accum_out=R[:, 0:1])
                nc.vector.scalar_tensor_tensor(
                    scr2[:], C2b[:, 0:32], 1.0, C2b[:, 0:32],
                    op0=ALU.mult, op1=ALU.mult, accum_out=r1s[:],
                )
                nc.vector.tensor_tensor(R[:, 1:2], r1s[:], C2b[:, 32:33],
                                        op=ALU.subtract)

            # num_graphs: rng = 100 / (max(batch) + 1); Vector is idle
            # here while Scalar finishes the exp accumulation
            ng = pool.tile([1, 1], F32, tag="ng")
            nc.vector.tensor_reduce(ng[:], ptb[0:1, 264:328], axis=AX.X, op=ALU.max)
            ng1 = pool.tile([1, 1], F32, tag="ng1")
            nc.vector.tensor_scalar(ng1[:], ng[:], 1.0, 0.01, op0=ALU.add, op1=ALU.mult)
            rng = pool.tile([1, 1], F32, tag="rng")
            nc.vector.reciprocal(rng[:], ng1[:])

            F = ppool.tile([1, 2], F32, tag="F")
            nc.tensor.matmul(F[:], ones_t[:], R[:], start=True, stop=True)

            l2 = pool.tile([1, 1], F32, tag="l2")
            SC = PENALTY_SCALE / N_NODES
            nc.vector.tensor_scalar(l2[:], F[:, 0:1], SC,
                                    -float(PAD_NODES) * SC,
                                    op0=ALU.mult, op1=ALU.add)
            # res = F1 * (100/ng) + l2 — F1 read straight from PSUM
            nc.vector.scalar_tensor_tensor(
                res, F[:, 1:2], rng[:], l2[:], op0=ALU.mult, op1=ALU.add
            )

    # post-TileContext output DMA overlaps the fixed epilogue tail
    # (gpsimd SWDGE trigger: cheap on-queue, gen runs async on the Q7)
    nc.gpsimd.dma_start(outd, res).then_inc(odma_sem, 16)

    nc.compile()
    return nc


def _pack_core(tt, uu, p, T):
    """Pack one core's edge shard: f32 payload [128, 4*T] plus int16
    local_scatter index columns [128, 2*T]."""
    ne = tt.shape[0]
    npad = T * 128

    def pad(a, fill):
        out = np.full(npad, fill, np.float64)
        out[:ne] = a
        return out.reshape(T, 128).T  # [128, T]

    self_loop = uu == tt
    tvec = np.arange(T, dtype=np.float64)[None, :]
    t_lo = pad(tt % 128, 0.0)
    t_hi = pad(tt // 128, float(HI))     # sentinel hi -> matches nothing
    pf = pad(p, 0.0)
    wf = pad(2.0 - self_loop, 0.0)       # d_e = 2 - m_e, 0 on pad slots
    ed = np.concatenate([t_lo, t_hi, pf, wf], axis=1).astype(np.float32)
    # local_scatter indices: -1 rows (self-loops, pads) stay zero
    u_lo = pad(uu % 128, 0.0)
    u_hi = pad(uu // 128, 0.0)
    dead = pad(np.where(self_loop, 1.0, 0.0), 1.0) > 0.5
    i_src = np.where(dead, -1.0, tvec * LO + u_lo)
    i_rsu = np.where(dead, -1.0, tvec * HI + u_hi)
    ei2 = np.concatenate([i_src, i_rsu], axis=1).astype(np.int16)
    return ed, ei2


_CACHE = {}


def _get(name, builder, *a):
    if name not in _CACHE:
        _CACHE[name] = builder(*a)
    return _CACHE[name]


def kernel(x, edge_index, edge_feature, batch, _trace=False):
    x = np.asarray(x)
    ei = np.asarray(edge_index).astype(np.int64)
    p = np.asarray(edge_feature).astype(np.float32)[:, 0]
    batch = np.asarray(batch).astype(np.int64)

    uu_all = ei[0].astype(np.float64)
    tt_all = ei[1].astype(np.float64)

    # ---- phase 1: per-core partials (no cross-core dependencies)
    nc1 = _get("p1", _build_phase1, TPC)
    in_maps = []
    for c in range(N_CORES):
        sl = slice(c * EPC, (c + 1) * EPC)
        ed, ei2 = _pack_core(tt_all[sl], uu_all[sl], p[sl], TPC)
        in_maps.append({"edata": ed, "eidx2": ei2})
    r1 = bass_utils.run_bass_kernel_spmd(
        nc1, in_maps, core_ids=list(range(N_CORES)), trace=_trace
    )

    # gather/unshard the per-core partials (pure data movement)
    parts = np.stack(
        [np.asarray(r1.results[c]["partial"]) for c in range(N_CORES)], axis=2
    ).astype(np.float16)                               # [p, x, c], c innermost

    # ---- phase 2: combine on one core
    nc2 = _get("p2", _build_phase2)
    btail = np.zeros((128, 64), np.float16)
    btail[0, :] = batch[-64:].astype(np.float16)
    partsa = parts[:, 0:32, :].reshape(128, 256)
    partsb = np.concatenate([parts[:, 32:65, :].reshape(128, 264), btail], axis=1)
    r2 = bass_utils.run_bass_kernel_spmd(
        nc2, [{"partsa": partsa, "partsb": partsb}], core_ids=[0], trace=_trace,
    )
    out = np.asarray(r2.results[0]["out"], dtype=np.float32).reshape(1, 1)
    if _trace:
        kernel.last_results = (r1, r2)
    return out
